# revision 10
# baseline (speedup 1.0000x reference)
"""Trainium2 Bass kernel for Llama-style GQA attention (T=2048, HID=4096,
H=32 q-heads, KV=8 kv-heads, D=128), tensor-parallel over heads on 8 cores.

Per-core work (core c):
  - QKV projection for its 4 q-heads + 1 kv-head (K and V) with RoPE fused
    into the PSUM drains.
  - Causal attention for its 4 heads, computed as scores^T [s, q] so that
    softmax-normalized P tiles feed the PV matmul directly (no transposes)
    and the PV output [d, q] is exactly the lhsT layout o_proj needs.
    Softmax skips the max-subtraction (scores are O(10), exp is safe in
    fp32) and gets denominators from a ones-stationary matmul.
  - Partial o_proj: attn^T(local heads) x Wo^T(local rows) -> [T, HID]
    partial sum.  Host adds the 8 partials (the "all-reduce").

All matmuls use the float32r dtype view (full PE rate for moving dim >=
256; plain fp32 runs at 1/4 rate).
"""

import numpy as np

import concourse.bass as bass
import concourse.bacc as bacc
import concourse.mybir as mybir
import concourse.tile as tile
from concourse import bass_utils
from concourse.masks import make_identity

T = 2048
HID = 4096
H = 32
KVH = 8
D = 128
NCORES = 8
HPC = H // NCORES          # q-heads per core = 4
THETA = 10000.0
F32 = mybir.dt.float32
F32R = mybir.dt.float32r
SCALE = float(D) ** -0.5

# QKV projection output blocks per core: 4 q-heads, 1 k-head, 1 v-head
NB = HPC + 2               # 6 blocks of 128
NQK = HPC + 1              # blocks 0..4 get RoPE (Q0..Q3, K); block 5 is V

TQ = 256                   # QKV t-chunk width (8 chunks)
NTQ = T // TQ
KCH = HID // 128           # 32 contraction chunks


def _pieces(lo, hi, step=512):
    """Split [lo, hi) at multiples of `step` (PSUM-bank-aligned chunks)."""
    out = []
    while lo < hi:
        nxt = min(hi, (lo // step + 1) * step)
        out.append((lo, nxt))
        lo = nxt
    return out


def build_nc():
    nc = bacc.Bacc("TRN2", target_bir_lowering=False, debug=False,
                   num_devices=NCORES)

    hT = nc.dram_tensor("hT", [HID, T], F32R, kind="ExternalInput").ap()
    wqkvT = nc.dram_tensor("wqkvT", [HID, NB * D], F32R, kind="ExternalInput").ap()
    woT = nc.dram_tensor("woT", [HPC * D, HID], F32R, kind="ExternalInput").ap()
    # cos2 = [cos; cos], sinm2 = [-sin; +sin] stacked along d (see host_inputs)
    cosT = nc.dram_tensor("cosT", [D, T], F32, kind="ExternalInput").ap()
    sinT = nc.dram_tensor("sinT", [D, T], F32, kind="ExternalInput").ap()
    trim = nc.dram_tensor("trim", [128, 128], F32R, kind="ExternalInput").ap()
    onesm = nc.dram_tensor("onesm", [128, 128], F32R, kind="ExternalInput").ap()
    out = nc.dram_tensor("out", [T, HID], F32, kind="ExternalOutput").ap()

    # DRAM scratch: roped q/k heads [5*128, T] and transposed V [T, 128]
    qk_dram = nc.dram_tensor("qk_dram", [NQK * D, T], F32R, kind="Internal").ap()
    v_dram = nc.dram_tensor("v_dram", [T, D], F32R, kind="Internal").ap()

    with tile.TileContext(nc) as tc:
        # ---------------- Phase 1: QKV projection + RoPE + V transpose ----
        with tc.tile_pool(name="qkvconst", bufs=1) as cpool, \
             tc.tile_pool(name="wq", bufs=1) as wpool, \
             tc.tile_pool(name="hid", bufs=2) as hpool, \
             tc.tile_pool(name="qkvstage", bufs=3) as spool, \
             tc.tile_pool(name="qkvpsum", bufs=2, space="PSUM") as qpsum, \
             tc.tile_pool(name="trpsum", bufs=2, space="PSUM") as tpsum:

            ident = cpool.tile([128, 128], F32)
            make_identity(nc, ident)
            cos_sb = cpool.tile([D, T], F32)
            sin_sb = cpool.tile([D, T], F32)
            for tq in range(NTQ):
                sl = slice(tq * TQ, (tq + 1) * TQ)
                nc.sync.dma_start(cos_sb[:, sl], cosT[:, sl])
                nc.sync.dma_start(sin_sb[:, sl], sinT[:, sl])

            wq = []
            for k in range(KCH):
                wt = wpool.tile([128, NB * D], F32R, name=f"wq{k}", tag=f"wq{k}")
                for nb in range(NB):
                    sl = slice(nb * D, (nb + 1) * D)
                    nc.sync.dma_start(wt[:, sl],
                                      wqkvT[k * 128:(k + 1) * 128, sl])
                wq.append(wt)

            for tq in range(NTQ):
                tlo = tq * TQ
                hk = []
                for k in range(KCH):
                    ht = hpool.tile([128, TQ], F32R, name=f"hk{k}", tag=f"hk{k}")
                    nc.sync.dma_start(ht, hT[k * 128:(k + 1) * 128, tlo:tlo + TQ])
                    hk.append(ht)
                for nb in range(NB):
                    ps = qpsum.tile([128, TQ], F32, name="qkv_ps", tag="qkv_ps")
                    for k in range(KCH):
                        nc.tensor.matmul(
                            ps, wq[k][:, nb * D:(nb + 1) * D], hk[k],
                            start=(k == 0), stop=(k == KCH - 1))
                    if nb < NQK:
                        # RoPE: out = raw*cos2 + swap_halves(raw)*[-sin;+sin]
                        # (DVE lanes are partition-fixed, so the half swap
                        # goes through an SBUF->SBUF DMA.)
                        raw = spool.tile([128, TQ], F32, name="roperaw",
                                         tag="roperaw")
                        nc.scalar.copy(raw, ps)
                        sw = spool.tile([128, TQ], F32, name="ropesw",
                                        tag="ropesw")
                        nc.sync.dma_start(sw[0:64], raw[64:128])
                        nc.sync.dma_start(sw[64:128], raw[0:64])
                        st = spool.tile([128, TQ], F32R, name="ropest",
                                        tag="ropest")
                        c_sl = cos_sb[:, tlo:tlo + TQ]
                        s_sl = sin_sb[:, tlo:tlo + TQ]
                        nc.vector.tensor_mul(st, raw, c_sl)
                        nc.vector.tensor_mul(sw, sw, s_sl)
                        nc.vector.tensor_add(st, st, sw)
                        nc.sync.dma_start(
                            qk_dram[nb * D:(nb + 1) * D, tlo:tlo + TQ], st)
                    else:
                        # V: drain, transpose 128x128 tiles, store [t, d]
                        vs = spool.tile([128, TQ], F32, name="vstage", tag="vstage")
                        nc.scalar.copy(vs, ps)
                        for i in range(TQ // 128):
                            tp = tpsum.tile([128, 128], F32, name="vt_ps",
                                            tag="vt_ps")
                            nc.tensor.transpose(tp, vs[:, i * 128:(i + 1) * 128],
                                                ident)
                            vt = spool.tile([128, 128], F32R, name="vt_sb",
                                            tag="vt_sb")
                            nc.vector.tensor_copy(vt, tp)
                            nc.sync.dma_start(
                                v_dram[tlo + i * 128:tlo + (i + 1) * 128, :], vt)

        # ---------------- Phase 2+3 SBUF residents ------------------------
        with tc.tile_pool(name="aconst", bufs=1) as apool, \
             tc.tile_pool(name="attn_out", bufs=1) as opool, \
             tc.tile_pool(name="wo", bufs=1) as wopool:

            tri = apool.tile([128, 128], F32R)
            nc.sync.dma_start(tri, trim)
            ones = apool.tile([128, 128], F32R)
            nc.sync.dma_start(ones, onesm)
            kt = apool.tile([D, T], F32R)
            for j in range(T // 128):
                sl = slice(j * 128, (j + 1) * 128)
                nc.sync.dma_start(kt[:, sl], qk_dram[HPC * D:(HPC + 1) * D, sl])
            vv = apool.tile([128, T], F32R)   # [:, j*128:+128] = V[j-block] [s,d]
            for j in range(T // 128):
                nc.sync.dma_start(vv[:, j * 128:(j + 1) * 128],
                                  v_dram[j * 128:(j + 1) * 128, :])

            wo = []
            for h in range(HPC):
                wt = wopool.tile([128, HID], F32R, name=f"wo{h}", tag=f"wo{h}")
                for cc in range(HID // 512):
                    sl = slice(cc * 512, (cc + 1) * 512)
                    nc.sync.dma_start(wt[:, sl], woT[h * D:(h + 1) * D, sl])
                wo.append(wt)

            attn = []
            for h in range(HPC):
                at = opool.tile([D, T], F32R, name=f"attn{h}", tag=f"attn{h}")
                attn.append(at)

            # ---------------- Phase 2: attention per head -----------------
            with tc.tile_pool(name="qt", bufs=2) as qtpool, \
                 tc.tile_pool(name="pj", bufs=1) as ppool, \
                 tc.tile_pool(name="rec", bufs=2) as rpool, \
                 tc.tile_pool(name="scps", bufs=2, space="PSUM") as scps, \
                 tc.tile_pool(name="pvps", bufs=2, space="PSUM") as pvps, \
                 tc.tile_pool(name="dnps", bufs=1, space="PSUM") as dnps:

                for h in range(HPC):
                    qt = qtpool.tile([D, T], F32R, name="qt", tag="qt")
                    for qq in range(T // 512):
                        sl = slice(qq * 512, (qq + 1) * 512)
                        nc.sync.dma_start(qt[:, sl],
                                          qk_dram[h * D:(h + 1) * D, sl])

                    for half in range(2):
                        q_lo = 1024 * half
                        js = range(8 * (half + 1))
                        pv = pvps.tile([128, 1024], F32, name="pv", tag="pv")
                        dn = dnps.tile([128, 1024], F32, name="dn", tag="dn")

                        pjs = {}
                        # scores + exp (+ causal mask on the diagonal block)
                        for j in js:
                            ls = max(0, 128 * j - q_lo)
                            pj = ppool.tile([128, 1024 - ls], F32R,
                                            name=f"p{j}", tag=f"p{j}")
                            pjs[j] = (pj, ls)
                            for (plo, phi) in _pieces(ls, 1024):
                                w = phi - plo
                                sc = scps.tile([128, 512], F32, name="sc",
                                               tag="sc")
                                nc.tensor.matmul(
                                    sc[:, :w],
                                    kt[:, j * 128:(j + 1) * 128],
                                    qt[:, q_lo + plo:q_lo + phi],
                                    start=True, stop=True)
                                nc.scalar.activation(
                                    pj[:, plo - ls:phi - ls], sc[:, :w],
                                    mybir.ActivationFunctionType.Exp,
                                    scale=SCALE)
                            if 128 * j >= q_lo:
                                nc.vector.tensor_mul(pj[:, 0:128],
                                                     pj[:, 0:128], tri)
                        # PV + denominator accumulation over j
                        for j in js:
                            pj, ls = pjs[j]
                            for (plo, phi) in _pieces(ls, 1024):
                                last = (q_lo + phi) // 128 - 1
                                kw = dict(start=(j == 0), stop=(j == last))
                                nc.tensor.matmul(
                                    pv[:, plo:phi],
                                    vv[:, j * 128:(j + 1) * 128],
                                    pj[:, plo - ls:phi - ls], **kw)
                                nc.tensor.matmul(
                                    dn[:, plo:phi], ones,
                                    pj[:, plo - ls:phi - ls], **kw)
                        rec = rpool.tile([128, 1024], F32, name="rec", tag="rec")
                        nc.vector.reciprocal(rec, dn)
                        nc.vector.tensor_mul(attn[h][:, q_lo:q_lo + 1024],
                                             pv, rec)

            # ---------------- Phase 3: o_proj partial ---------------------
            with tc.tile_pool(name="ostage", bufs=2) as ospool, \
                 tc.tile_pool(name="ops", bufs=2, space="PSUM") as opsum:
                for tb in range(T // 128):
                    for ch in range(2):
                        ps = opsum.tile([128, 2048], F32, name="o_ps",
                                        tag="o_ps")
                        for h in range(HPC):
                            lhs = attn[h][:, tb * 128:(tb + 1) * 128]
                            for cc in range(4):
                                cl = ch * 2048 + cc * 512
                                nc.tensor.matmul(
                                    ps[:, cc * 512:(cc + 1) * 512],
                                    lhs, wo[h][:, cl:cl + 512],
                                    start=(h == 0), stop=(h == HPC - 1))
                        ob = ospool.tile([128, 2048], F32, name="ob", tag="ob")
                        nc.scalar.copy(ob, ps)
                        for cc in range(4):
                            nc.sync.dma_start(
                                out[tb * 128:(tb + 1) * 128,
                                    ch * 2048 + cc * 512:
                                    ch * 2048 + (cc + 1) * 512],
                                ob[:, cc * 512:(cc + 1) * 512])
    nc.compile()
    return nc


def host_inputs(hidden_states, positions, Wqkv, Wo):
    """Build the 8 per-core input maps (host-side sharding + layout prep)."""
    f = np.float32
    hT = np.ascontiguousarray(hidden_states.T.astype(f))
    half = D // 2
    inv_freq = 1.0 / (THETA ** (np.arange(half, dtype=np.float64) / half))
    ang = inv_freq[:, None] * positions.astype(np.float64)[None, :]
    cos = np.cos(ang).astype(f)
    sin = np.sin(ang).astype(f)
    cosT = np.vstack([cos, cos])                  # [D, T]
    sinT = np.vstack([-sin, sin])                 # rotate-half sign baked in
    trim = (np.arange(128)[:, None] <= np.arange(128)[None, :]).astype(f)

    in_maps = []
    for c in range(NCORES):
        rows = list(range(c * HPC * D, (c + 1) * HPC * D))          # Q heads
        rows += list(range(H * D + c * D, H * D + (c + 1) * D))     # K head
        rows += list(range((H + KVH) * D + c * D,
                           (H + KVH) * D + (c + 1) * D))            # V head
        wqkvT = np.ascontiguousarray(Wqkv[rows, :].T.astype(f))
        woT = np.ascontiguousarray(Wo[:, c * HPC * D:(c + 1) * HPC * D].T
                                   .astype(f))
        in_maps.append({"hT": hT, "wqkvT": wqkvT, "woT": woT,
                        "cosT": cosT, "sinT": sinT, "trim": trim,
                        "onesm": np.ones((128, 128), f)})
    return in_maps


_NC_CACHE = {}


def get_nc():
    if "nc" not in _NC_CACHE:
        _NC_CACHE["nc"] = build_nc()
    return _NC_CACHE["nc"]


def kernel(hidden_states, positions, Wqkv, Wo, _trace=False):
    hidden_states = np.asarray(hidden_states)
    positions = np.asarray(positions)
    Wqkv = np.asarray(Wqkv)
    Wo = np.asarray(Wo)
    in_maps = host_inputs(hidden_states, positions, Wqkv, Wo)
    nc = get_nc()
    res = bass_utils.run_bass_kernel_spmd(
        nc, in_maps, core_ids=list(range(NCORES)), trace=_trace)
    acc = np.zeros((T, HID), np.float64)
    for r in res.results:
        acc += r["out"].astype(np.float64)
    out = acc.astype(np.float32)
    if _trace:
        return out, res
    return out


# revision 11
# speedup vs baseline: 175.0574x; 175.0574x over previous
"""Trainium2 Bass kernel for Llama-style GQA attention (T=2048, HID=4096,
H=32 q-heads, KV=8 kv-heads, D=128), tensor-parallel over heads on 8 cores.

Per-core work (core c):
  - QKV projection for its 4 q-heads + 1 kv-head (K and V) with RoPE fused
    into the PSUM drains.
  - Causal attention for its 4 heads, computed as scores^T [s, q] so that
    softmax-normalized P tiles feed the PV matmul directly (no transposes)
    and the PV output [d, q] is exactly the lhsT layout o_proj needs.
    Softmax skips the max-subtraction (scores are O(10), exp is safe in
    fp32) and gets denominators from a ones-stationary matmul.
  - Partial o_proj: attn^T(local heads) x Wo^T(local rows) -> [T, HID]
    partial sum.  Host adds the 8 partials (the "all-reduce").

All matmuls use the float32r dtype view (full PE rate for moving dim >=
256; plain fp32 runs at 1/4 rate).
"""

import numpy as np

import concourse.bass as bass
import concourse.bacc as bacc
import concourse.mybir as mybir
import concourse.tile as tile
from concourse import bass_utils
from concourse.masks import make_identity

T = 2048
HID = 4096
H = 32
KVH = 8
D = 128
NCORES = 8
HPC = H // NCORES          # q-heads per core = 4
THETA = 10000.0
F32 = mybir.dt.float32
F32R = mybir.dt.float32r
SCALE = float(D) ** -0.5

# QKV projection output blocks per core: 4 q-heads, 1 k-head, 1 v-head
NB = HPC + 2               # 6 blocks of 128
NQK = HPC + 1              # blocks 0..4 get RoPE (Q0..Q3, K); block 5 is V

TQ = 256                   # QKV t-chunk width (8 chunks)
NTQ = T // TQ
KCH = HID // 128           # 32 contraction chunks


def _pieces(lo, hi, step=512):
    """Split [lo, hi) at multiples of `step` (PSUM-bank-aligned chunks)."""
    out = []
    while lo < hi:
        nxt = min(hi, (lo // step + 1) * step)
        out.append((lo, nxt))
        lo = nxt
    return out


def build_nc(loop_n=1):
    nc = bacc.Bacc("TRN2", target_bir_lowering=False, debug=False,
                   num_devices=NCORES)

    hT = nc.dram_tensor("hT", [HID, T], F32R, kind="ExternalInput").ap()
    wqkvT = nc.dram_tensor("wqkvT", [HID, NB * D], F32R, kind="ExternalInput").ap()
    woT = nc.dram_tensor("woT", [HPC * D, HID], F32R, kind="ExternalInput").ap()
    # cos2 = [cos; cos], sinm2 = [-sin; +sin] stacked along d (see host_inputs)
    cosT = nc.dram_tensor("cosT", [D, T], F32, kind="ExternalInput").ap()
    sinT = nc.dram_tensor("sinT", [D, T], F32, kind="ExternalInput").ap()
    trim = nc.dram_tensor("trim", [128, 128], F32R, kind="ExternalInput").ap()
    onesm = nc.dram_tensor("onesm", [128, 128], F32R, kind="ExternalInput").ap()
    out = nc.dram_tensor("out", [T, HID], F32, kind="ExternalOutput").ap()

    # DRAM scratch: roped q/k heads [5*128, T] and transposed V [T, 128]
    qk_dram = nc.dram_tensor("qk_dram", [NQK * D, T], F32R, kind="Internal").ap()
    v_dram = nc.dram_tensor("v_dram", [T, D], F32R, kind="Internal").ap()

    import contextlib

    with tile.TileContext(nc) as tc, contextlib.ExitStack() as _loopctx:
        if loop_n > 1:
            _loopctx.enter_context(tc.For_i(0, loop_n))
        # ---------------- Phase 1: QKV projection + RoPE + V transpose ----
        with tc.tile_pool(name="qkvconst", bufs=1) as cpool, \
             tc.tile_pool(name="wq", bufs=1) as wpool, \
             tc.tile_pool(name="hid", bufs=2) as hpool, \
             tc.tile_pool(name="qkvstage", bufs=3) as spool, \
             tc.tile_pool(name="qkvpsum", bufs=2, space="PSUM") as qpsum, \
             tc.tile_pool(name="trpsum", bufs=2, space="PSUM") as tpsum:

            ident = cpool.tile([128, 128], F32)
            make_identity(nc, ident)
            cos_sb = cpool.tile([D, T], F32)
            sin_sb = cpool.tile([D, T], F32)
            for tq in range(NTQ):
                sl = slice(tq * TQ, (tq + 1) * TQ)
                nc.sync.dma_start(cos_sb[:, sl], cosT[:, sl])
                nc.sync.dma_start(sin_sb[:, sl], sinT[:, sl])

            wq = []
            for k in range(KCH):
                wt = wpool.tile([128, NB * D], F32R, name=f"wq{k}", tag=f"wq{k}")
                for nb in range(NB):
                    sl = slice(nb * D, (nb + 1) * D)
                    nc.sync.dma_start(wt[:, sl],
                                      wqkvT[k * 128:(k + 1) * 128, sl])
                wq.append(wt)

            for tq in range(NTQ):
                tlo = tq * TQ
                hk = []
                for k in range(KCH):
                    ht = hpool.tile([128, TQ], F32R, name=f"hk{k}", tag=f"hk{k}")
                    nc.sync.dma_start(ht, hT[k * 128:(k + 1) * 128, tlo:tlo + TQ])
                    hk.append(ht)
                for nb in range(NB):
                    ps = qpsum.tile([128, TQ], F32, name="qkv_ps", tag="qkv_ps")
                    for k in range(KCH):
                        nc.tensor.matmul(
                            ps, wq[k][:, nb * D:(nb + 1) * D], hk[k],
                            start=(k == 0), stop=(k == KCH - 1))
                    if nb < NQK:
                        # RoPE: out = raw*cos2 + swap_halves(raw)*[-sin;+sin]
                        # (DVE lanes are partition-fixed, so the half swap
                        # goes through an SBUF->SBUF DMA.)
                        raw = spool.tile([128, TQ], F32, name="roperaw",
                                         tag="roperaw")
                        nc.scalar.copy(raw, ps)
                        sw = spool.tile([128, TQ], F32, name="ropesw",
                                        tag="ropesw")
                        nc.sync.dma_start(sw[0:64], raw[64:128])
                        nc.sync.dma_start(sw[64:128], raw[0:64])
                        st = spool.tile([128, TQ], F32R, name="ropest",
                                        tag="ropest")
                        c_sl = cos_sb[:, tlo:tlo + TQ]
                        s_sl = sin_sb[:, tlo:tlo + TQ]
                        nc.vector.tensor_mul(st, raw, c_sl)
                        nc.vector.tensor_mul(sw, sw, s_sl)
                        nc.vector.tensor_add(st, st, sw)
                        nc.sync.dma_start(
                            qk_dram[nb * D:(nb + 1) * D, tlo:tlo + TQ], st)
                    else:
                        # V: drain, transpose 128x128 tiles, store [t, d]
                        vs = spool.tile([128, TQ], F32, name="vstage", tag="vstage")
                        nc.scalar.copy(vs, ps)
                        for i in range(TQ // 128):
                            tp = tpsum.tile([128, 128], F32, name="vt_ps",
                                            tag="vt_ps")
                            nc.tensor.transpose(tp, vs[:, i * 128:(i + 1) * 128],
                                                ident)
                            vt = spool.tile([128, 128], F32R, name="vt_sb",
                                            tag="vt_sb")
                            nc.vector.tensor_copy(vt, tp)
                            nc.sync.dma_start(
                                v_dram[tlo + i * 128:tlo + (i + 1) * 128, :], vt)

        # ---------------- Phase 2+3 SBUF residents ------------------------
        with tc.tile_pool(name="aconst", bufs=1) as apool, \
             tc.tile_pool(name="attn_out", bufs=1) as opool, \
             tc.tile_pool(name="wo", bufs=1) as wopool:

            tri = apool.tile([128, 128], F32R)
            nc.sync.dma_start(tri, trim)
            ones = apool.tile([128, 128], F32R)
            nc.sync.dma_start(ones, onesm)
            kt = apool.tile([D, T], F32R)
            for j in range(T // 128):
                sl = slice(j * 128, (j + 1) * 128)
                nc.sync.dma_start(kt[:, sl], qk_dram[HPC * D:(HPC + 1) * D, sl])
            vv = apool.tile([128, T], F32R)   # [:, j*128:+128] = V[j-block] [s,d]
            for j in range(T // 128):
                nc.sync.dma_start(vv[:, j * 128:(j + 1) * 128],
                                  v_dram[j * 128:(j + 1) * 128, :])

            wo = []
            for h in range(HPC):
                wt = wopool.tile([128, HID], F32R, name=f"wo{h}", tag=f"wo{h}")
                for cc in range(HID // 512):
                    sl = slice(cc * 512, (cc + 1) * 512)
                    nc.sync.dma_start(wt[:, sl], woT[h * D:(h + 1) * D, sl])
                wo.append(wt)

            attn = []
            for h in range(HPC):
                at = opool.tile([D, T], F32R, name=f"attn{h}", tag=f"attn{h}")
                attn.append(at)

            # ---------------- Phase 2: attention per head -----------------
            with tc.tile_pool(name="qt", bufs=2) as qtpool, \
                 tc.tile_pool(name="pj", bufs=1) as ppool, \
                 tc.tile_pool(name="rec", bufs=2) as rpool, \
                 tc.tile_pool(name="scps", bufs=2, space="PSUM") as scps, \
                 tc.tile_pool(name="pvps", bufs=2, space="PSUM") as pvps, \
                 tc.tile_pool(name="dnps", bufs=1, space="PSUM") as dnps:

                for h in range(HPC):
                    qt = qtpool.tile([D, T], F32R, name="qt", tag="qt")
                    for qq in range(T // 512):
                        sl = slice(qq * 512, (qq + 1) * 512)
                        nc.sync.dma_start(qt[:, sl],
                                          qk_dram[h * D:(h + 1) * D, sl])

                    for half in range(2):
                        q_lo = 1024 * half
                        js = range(8 * (half + 1))
                        pv = pvps.tile([128, 1024], F32, name="pv", tag="pv")
                        dn = dnps.tile([128, 1024], F32, name="dn", tag="dn")

                        pjs = {}
                        # scores + exp (+ causal mask on the diagonal block)
                        for j in js:
                            ls = max(0, 128 * j - q_lo)
                            pj = ppool.tile([128, 1024 - ls], F32R,
                                            name=f"p{j}", tag=f"p{j}")
                            pjs[j] = (pj, ls)
                            for (plo, phi) in _pieces(ls, 1024):
                                w = phi - plo
                                sc = scps.tile([128, 512], F32, name="sc",
                                               tag="sc")
                                nc.tensor.matmul(
                                    sc[:, :w],
                                    kt[:, j * 128:(j + 1) * 128],
                                    qt[:, q_lo + plo:q_lo + phi],
                                    start=True, stop=True)
                                nc.scalar.activation(
                                    pj[:, plo - ls:phi - ls], sc[:, :w],
                                    mybir.ActivationFunctionType.Exp,
                                    scale=SCALE)
                            if 128 * j >= q_lo:
                                nc.vector.tensor_mul(pj[:, 0:128],
                                                     pj[:, 0:128], tri)
                        # PV + denominator accumulation over j
                        for j in js:
                            pj, ls = pjs[j]
                            for (plo, phi) in _pieces(ls, 1024):
                                last = (q_lo + phi) // 128 - 1
                                kw = dict(start=(j == 0), stop=(j == last))
                                nc.tensor.matmul(
                                    pv[:, plo:phi],
                                    vv[:, j * 128:(j + 1) * 128],
                                    pj[:, plo - ls:phi - ls], **kw)
                                nc.tensor.matmul(
                                    dn[:, plo:phi], ones,
                                    pj[:, plo - ls:phi - ls], **kw)
                        rec = rpool.tile([128, 1024], F32, name="rec", tag="rec")
                        nc.vector.reciprocal(rec, dn)
                        nc.vector.tensor_mul(attn[h][:, q_lo:q_lo + 1024],
                                             pv, rec)

            # ---------------- Phase 3: o_proj partial ---------------------
            with tc.tile_pool(name="ostage", bufs=2) as ospool, \
                 tc.tile_pool(name="ops", bufs=2, space="PSUM") as opsum:
                for tb in range(T // 128):
                    for ch in range(2):
                        ps = opsum.tile([128, 2048], F32, name="o_ps",
                                        tag="o_ps")
                        for h in range(HPC):
                            lhs = attn[h][:, tb * 128:(tb + 1) * 128]
                            for cc in range(4):
                                cl = ch * 2048 + cc * 512
                                nc.tensor.matmul(
                                    ps[:, cc * 512:(cc + 1) * 512],
                                    lhs, wo[h][:, cl:cl + 512],
                                    start=(h == 0), stop=(h == HPC - 1))
                        ob = ospool.tile([128, 2048], F32, name="ob", tag="ob")
                        nc.scalar.copy(ob, ps)
                        for cc in range(4):
                            nc.sync.dma_start(
                                out[tb * 128:(tb + 1) * 128,
                                    ch * 2048 + cc * 512:
                                    ch * 2048 + (cc + 1) * 512],
                                ob[:, cc * 512:(cc + 1) * 512])
    nc.compile()
    return nc


def host_inputs(hidden_states, positions, Wqkv, Wo):
    """Build the 8 per-core input maps (host-side sharding + layout prep)."""
    f = np.float32
    hT = np.ascontiguousarray(hidden_states.T.astype(f))
    half = D // 2
    inv_freq = 1.0 / (THETA ** (np.arange(half, dtype=np.float64) / half))
    ang = inv_freq[:, None] * positions.astype(np.float64)[None, :]
    cos = np.cos(ang).astype(f)
    sin = np.sin(ang).astype(f)
    cosT = np.vstack([cos, cos])                  # [D, T]
    sinT = np.vstack([-sin, sin])                 # rotate-half sign baked in
    trim = (np.arange(128)[:, None] <= np.arange(128)[None, :]).astype(f)

    in_maps = []
    for c in range(NCORES):
        rows = list(range(c * HPC * D, (c + 1) * HPC * D))          # Q heads
        rows += list(range(H * D + c * D, H * D + (c + 1) * D))     # K head
        rows += list(range((H + KVH) * D + c * D,
                           (H + KVH) * D + (c + 1) * D))            # V head
        wqkvT = np.ascontiguousarray(Wqkv[rows, :].T.astype(f))
        woT = np.ascontiguousarray(Wo[:, c * HPC * D:(c + 1) * HPC * D].T
                                   .astype(f))
        in_maps.append({"hT": hT, "wqkvT": wqkvT, "woT": woT,
                        "cosT": cosT, "sinT": sinT, "trim": trim,
                        "onesm": np.ones((128, 128), f)})
    return in_maps


_NC_CACHE = {}


def get_nc(loop_n=1):
    if loop_n not in _NC_CACHE:
        _NC_CACHE[loop_n] = build_nc(loop_n)
    return _NC_CACHE[loop_n]


def kernel(hidden_states, positions, Wqkv, Wo, _trace=False):
    hidden_states = np.asarray(hidden_states)
    positions = np.asarray(positions)
    Wqkv = np.asarray(Wqkv)
    Wo = np.asarray(Wo)
    in_maps = host_inputs(hidden_states, positions, Wqkv, Wo)
    nc = get_nc()
    res = bass_utils.run_bass_kernel_spmd(
        nc, in_maps, core_ids=list(range(NCORES)), trace=_trace)
    acc = np.zeros((T, HID), np.float64)
    for r in res.results:
        acc += r["out"].astype(np.float64)
    out = acc.astype(np.float32)
    if _trace:
        return out, res
    return out
